# revision 8
# baseline (speedup 1.0000x reference)
"""Trainium2 Bass kernel for nn_Net_43052752175597 (2-layer GraphSAGE, aggr=add).

reference:
    A_hat = (A != 0).T with unit diagonal          # [N, N], binary
    h1   = X @ W1;  agg1 = A_hat @ h1 + b1;  x1 = relu(l2norm(agg1))
    h2   = x1 @ W2; agg2 = A_hat @ h2 + b2;  out = l2norm(l2norm(agg2))

Sharding: row-shard A_hat (output nodes) across 8 cores. Each core owns a
contiguous block of 1280 padded nodes (N padded 10000 -> 10240). Per layer the
core computes h for its own nodes in fp16, AllGathers the transformed
features in two chunks (so the collectives and the post-collective SBUF
loads overlap the aggregation matmuls), then aggregates its A_hat row-slice
(fp8 tiles, SBUF-resident across both layers) against the gathered features.

The gathered-feature SBUF buffer is double-buffered across layers so layer-2
chunks can land while layer-1 aggregation still reads the layer-1 features.
The global source-node (k) order is permuted chunk-major/rank-major to match
the chunked AllGather output layout; the host-side A_hat^T tiling applies the
same permutation, so on-device indexing stays trivial.

Precision: A is exactly 0/1 so fp8 A tiles are exact. h flows as fp16
(2^-11 rounding) with f32 PSUM accumulation -> ~1e-4 final rel err.
"""

import sys

sys.path.insert(0, "/opt/trn_rl_repo")

import numpy as np
import ml_dtypes

import concourse.bass as bass
import concourse.tile as tile
from concourse import bacc, mybir
from concourse import bass_utils

N = 10000
NP = 10240          # padded node count
F = 256             # input feature dim
H = 128             # hidden dim
N_CORES = 8
PER_CORE = NP // N_CORES        # 1280 nodes per core
M_TILES = PER_CORE // 128       # 10
K_TILES = NP // 128             # 80
# AllGather chunk geometry (shared by both layers; fixes the t_hfull slot
# layout). Asymmetric: a big chunk fired one m-tile early, then a tiny tail
# chunk so the seam between layers exposes only a minimal collective.
CHUNK_BOUNDS = [0, 9, M_TILES]          # m-tile ranges [0,9) and [9,10)
N_CHUNKS = len(CHUNK_BOUNDS) - 1
CHUNK_SIZES = [CHUNK_BOUNDS[i + 1] - CHUNK_BOUNDS[i] for i in range(N_CHUNKS)]
SLOT_BASE = [N_CORES * sum(CHUNK_SIZES[:i]) for i in range(N_CHUNKS)]

AGG_MODE = "fp16"

_CACHE = {}


def _build_nc(agg_mode=AGG_MODE, single_core=False, compile=True, repeats=1):
    """Build + compile the 8-core SPMD Bass kernel. Returns the Bacc object.

    single_core=True builds a 1-core variant with the collectives replaced by
    equivalent-byte local DMAs — only for TimelineSim cost-model profiling.
    repeats>1 runs the whole 2-layer body N times (benchmarking only).
    """
    fp32 = mybir.dt.float32
    fp16 = mybir.dt.float16
    fp8 = mybir.dt.float8e4

    nc = bacc.Bacc(
        "TRN2",
        target_bir_lowering=False,
        debug=False,
        enable_asserts=True,
        num_devices=1 if single_core else N_CORES,
    )

    # Per-core external inputs
    a_pre = nc.dram_tensor("a_pre", [M_TILES, 128, K_TILES, 128], fp8,
                           kind="ExternalInput").ap()
    xt = nc.dram_tensor("xt", [128, 2, PER_CORE], fp16,
                        kind="ExternalInput").ap()
    w1 = nc.dram_tensor("w1", [128, 2, H], fp16, kind="ExternalInput").ap()
    w2 = nc.dram_tensor("w2", [128, H], fp16, kind="ExternalInput").ap()
    b1 = nc.dram_tensor("b1", [128, H], fp32, kind="ExternalInput").ap()
    b2 = nc.dram_tensor("b2", [128, H], fp32, kind="ExternalInput").ap()
    ident = nc.dram_tensor("ident", [128, 128], fp32, kind="ExternalInput").ap()
    out = nc.dram_tensor("out", [PER_CORE, H], fp32, kind="ExternalOutput").ap()

    with tile.TileContext(nc) as tc:
        with tc.tile_pool(name="const", bufs=1) as cpool, \
             tc.tile_pool(name="hfull", bufs=2) as hpool, \
             tc.tile_pool(name="acol", bufs=1) as apool, \
             tc.tile_pool(name="work", bufs=1) as wpool, \
             tc.tile_pool(name="shard", bufs=2) as spool, \
             tc.tile_pool(name="psum_agg", bufs=2, space="PSUM") as pagg, \
             tc.tile_pool(name="psum_aux", bufs=2, space="PSUM") as paux, \
             tc.tile_pool(name="psum_tr", bufs=2, space="PSUM") as ptr, \
             tc.tile_pool(name="dram", bufs=2, space="DRAM") as dpool:

            # ---- constants into SBUF (gather-critical path first) ----
            t_xt = cpool.tile([128, 2, PER_CORE], fp16)
            t_w1 = cpool.tile([128, 2, H], fp16)
            t_w2 = cpool.tile([128, H], fp16)
            t_b1 = cpool.tile([128, H], fp32)
            t_b2 = cpool.tile([128, H], fp32)
            t_id = cpool.tile([128, 128], fp32)
            # split so the first h1 m-tiles can start before the whole X^T
            # slice has landed
            nc.sync.dma_start(t_xt[:, :, 0:PER_CORE // 2],
                              xt[:, :, 0:PER_CORE // 2])
            nc.sync.dma_start(t_xt[:, :, PER_CORE // 2:],
                              xt[:, :, PER_CORE // 2:])
            nc.sync.dma_start(t_w1[:], w1[:])
            nc.sync.dma_start(t_w2[:], w2[:])
            nc.sync.dma_start(t_b1[:], b1[:])
            nc.sync.dma_start(t_b2[:], b2[:])
            nc.sync.dma_start(t_id[:], ident[:])

            # Whole per-core A slice stays resident in SBUF, loaded once
            # (m-major so early m-tiles can start ASAP) and reused by both
            # layers. SWDGE so these bulk loads don't queue ahead of the
            # latency-critical HWDGE gather/feature DMAs.
            t_acache = apool.tile([128, M_TILES, K_TILES, 128], fp8)
            for m in range(M_TILES):
                nc.gpsimd.dma_start(t_acache[:, m, :, :], a_pre[m])

            # per-layer working tiles
            t_agg = wpool.tile([128, M_TILES, H], fp32, tag="agg")
            t_x1 = wpool.tile([128, M_TILES, H], fp32)
            t_x1t = wpool.tile([128, M_TILES, H], fp16)
            t_ssq = wpool.tile([128, M_TILES], fp32, tag="ssq")
            t_nrm = wpool.tile([128, M_TILES], fp32, tag="nrm")
            t_inv = wpool.tile([128, M_TILES], fp32, tag="inv")
            t_sq_scratch = wpool.tile([128, H], fp32)
            t_outf = wpool.tile([128, M_TILES, H], fp32)

            def gather_chunk(h_shard, ck, t_hfull, tag):
                """AllGather own m-tiles [CHUNK_BOUNDS[ck], CHUNK_BOUNDS[ck+1])
                of h_shard into t_hfull slots starting at SLOT_BASE[ck]
                (rank-major layout within the chunk)."""
                mc = CHUNK_SIZES[ck]
                kc = N_CORES * mc
                base = SLOT_BASE[ck]
                msl = slice(CHUNK_BOUNDS[ck], CHUNK_BOUNDS[ck + 1])
                sh_dram = dpool.tile([mc * 128, H], fp16, tag=f"sh_dram{ck}",
                                     name=f"sh_{tag}")
                g_dram = dpool.tile([kc * 128, H], fp16,
                                    tag=f"g_dram{ck}", name=f"g_{tag}",
                                    addr_space="Shared")
                nc.sync.dma_start(
                    sh_dram[:].rearrange("(m p) f -> p m f", p=128),
                    h_shard[:, msl, :])
                if single_core:
                    for r in range(N_CORES):
                        ks = base + r * mc
                        nc.sync.dma_start(
                            t_hfull[:, ks:ks + mc, :],
                            sh_dram[:].rearrange("(m p) f -> p m f", p=128))
                else:
                    nc.gpsimd.collective_compute(
                        "AllGather", mybir.AluOpType.bypass,
                        replica_groups=[list(range(N_CORES))],
                        ins=[sh_dram.opt()], outs=[g_dram.opt()],
                    )
                    # split the SBUF load so aggregation can start on the
                    # first ranks' k-tiles while the rest still loads
                    gv = g_dram[:].rearrange("(k p) f -> p k f", p=128)
                    kh = kc // 2 if kc > 16 else kc
                    nc.sync.dma_start(
                        t_hfull[:, base:base + kh, :], gv[:, 0:kh, :])
                    if kh < kc:
                        nc.sync.dma_start(
                            t_hfull[:, base + kh:base + kc, :],
                            gv[:, kh:kc, :])

            def aggregate(m, t_bias, t_hfull):
                """t_agg[:, m, :] = A_hat-slice @ h + bias, plus row sum-sq."""
                ps = pagg.tile([128, H], fp32, tag="ps_agg")
                for k in range(K_TILES):
                    nc.tensor.matmul(ps[:], t_acache[:, m, k, :],
                                     t_hfull[:, k, :],
                                     start=(k == 0), stop=(k == K_TILES - 1))
                nc.vector.tensor_tensor(t_agg[:, m, :], ps[:, 0:H], t_bias[:],
                                        op=mybir.AluOpType.add)
                nc.scalar.activation(t_sq_scratch[:], t_agg[:, m, :],
                                     mybir.ActivationFunctionType.Square,
                                     accum_out=t_ssq[:, m:m + 1])

            for _rep in range(repeats):
              t_h1full = hpool.tile([128, K_TILES, H], fp16, tag="hfull",
                                    name=f"h1full_{_rep}")
              t_h2full = hpool.tile([128, K_TILES, H], fp16, tag="hfull",
                                    name=f"h2full_{_rep}")

              # =============== Layer 1: h1 = fp16(X @ W1) for own nodes ====
              h1_shard = spool.tile([128, M_TILES, H], fp16, tag="h_shard",
                                    name=f"h1_shard_{_rep}")
              for m in range(M_TILES):
                ps = paux.tile([128, H], fp32, tag="ps_h")
                for k in range(2):
                    nc.tensor.matmul(ps[:],
                                     t_xt[:, k, m * 128:(m + 1) * 128],
                                     t_w1[:, k, :],
                                     start=(k == 0), stop=(k == 1))
                nc.vector.tensor_copy(h1_shard[:, m, :], ps[:])
                for ck in range(N_CHUNKS):
                    if m == CHUNK_BOUNDS[ck + 1] - 1:
                        gather_chunk(h1_shard, ck, t_h1full,
                                     f"h1c{ck}_{_rep}")

              # ====== Layer 1 aggregation fused with norm/relu/h2 per m-tile.
              # The PE-side tail (transpose + h2 matmul) is software-pipelined
              # one m-tile behind the aggregation so the PE never waits on the
              # serial norm chain.
              h2_shard = spool.tile([128, M_TILES, H], fp16, tag="h_shard",
                                    name=f"h2_shard_{_rep}")

              def pe_tail(m):
                """transpose x1 tile -> fp16 stationary; h2 = x1t^T @ W2."""
                pst = ptr.tile([128, 128], fp32, tag="pst")
                nc.tensor.transpose(pst[:], t_x1[:, m, :], t_id[:])
                nc.vector.tensor_copy(t_x1t[:, m, :], pst[:])
                ps2 = paux.tile([128, H], fp32, tag="ps_h")
                nc.tensor.matmul(ps2[:], t_x1t[:, m, :], t_w2[:],
                                 start=True, stop=True)
                nc.vector.tensor_copy(h2_shard[:, m, :], ps2[:])
                for ck in range(N_CHUNKS):
                    if m == CHUNK_BOUNDS[ck + 1] - 1:
                        gather_chunk(h2_shard, ck, t_h2full,
                                     f"h2c{ck}_{_rep}")

              for m in range(M_TILES):
                aggregate(m, t_b1, t_h1full)
                msl = slice(m, m + 1)
                nc.scalar.sqrt(t_nrm[:, msl], t_ssq[:, msl])
                nc.vector.tensor_scalar_max(t_nrm[:, msl], t_nrm[:, msl], 1e-12)
                nc.vector.reciprocal(t_inv[:, msl], t_nrm[:, msl])
                # x1 = relu(agg1 / max(||agg1||, 1e-12))
                nc.scalar.activation(t_x1[:, m, :], t_agg[:, m, :],
                                     mybir.ActivationFunctionType.Relu,
                                     scale=t_inv[:, msl])
                if m >= 1:
                    pe_tail(m - 1)
              pe_tail(M_TILES - 1)

              # ====== Layer 2 aggregation fused with the final l2norm.
              # l2norm(l2norm(x)) == l2norm(x) up to f32 rounding whenever
              # ||x|| > eps (always: agg2 includes the b2 offset), since the
              # inner normalize yields a unit-norm vector.
              for m in range(M_TILES):
                aggregate(m, t_b2, t_h2full)
                msl = slice(m, m + 1)
                nc.scalar.sqrt(t_nrm[:, msl], t_ssq[:, msl])
                nc.vector.tensor_scalar_max(t_nrm[:, msl], t_nrm[:, msl], 1e-12)
                nc.vector.reciprocal(t_inv[:, msl], t_nrm[:, msl])
                nc.scalar.activation(t_outf[:, m, :], t_agg[:, m, :],
                                     mybir.ActivationFunctionType.Copy,
                                     scale=t_inv[:, msl])
                nc.sync.dma_start(
                    out[:].rearrange("(mm p) f -> p mm f", p=128)[:, m, :],
                    t_outf[:, m, :])

    if compile:
        nc.compile()
    return nc


def _k_perm():
    """New k-tile order: chunk-major, rank-major, tile-minor.
    perm[new_k] = old_k = rank*M_TILES + (chunk m-range start) + t."""
    perm = np.empty(K_TILES, dtype=np.int64)
    for ck in range(N_CHUNKS):
        mc = CHUNK_SIZES[ck]
        for r in range(N_CORES):
            for t in range(mc):
                perm[SLOT_BASE[ck] + r * mc + t] = \
                    r * M_TILES + CHUNK_BOUNDS[ck] + t
    return perm


def _prep_inputs(X, A, W1, b1, W2, b2, agg_mode=AGG_MODE):
    """Host-side sharding/layout prep. Returns in_maps for the 8 cores."""
    f32 = np.float32
    fp16 = np.float16
    a_np_dt = ml_dtypes.float8_e4m3

    # --- A_hat^T = binarized A with unit diagonal, padded, tiled ---
    Ab = np.zeros((NP, NP), dtype=a_np_dt)
    Ab[:N, :N] = (A != 0)
    idx = np.arange(N)
    Ab[idx, idx] = 1.0
    # [k, p, cm, n] -> [cm, p, k, n], then permute k to the chunked layout
    T = Ab.reshape(K_TILES, 128, K_TILES, 128).transpose(2, 1, 0, 3)
    T = T[:, :, _k_perm(), :]

    # --- X^T fp16, padded ---
    Xp = np.zeros((NP, F), dtype=f32)
    Xp[:N] = np.asarray(X, dtype=f32)
    XT = np.ascontiguousarray(Xp.T).astype(fp16)     # [256, NP]

    w1_host = np.ascontiguousarray(
        np.asarray(W1, dtype=f32).reshape(2, 128, H).transpose(1, 0, 2)
    ).astype(fp16)                                   # [128, 2, H]
    w2_host = np.asarray(W2, dtype=f32).astype(fp16)  # [128, H]

    b1_host = np.ascontiguousarray(
        np.broadcast_to(np.asarray(b1, dtype=f32), (128, H)))
    b2_host = np.ascontiguousarray(
        np.broadcast_to(np.asarray(b2, dtype=f32), (128, H)))
    ident = np.eye(128, dtype=f32)

    in_maps = []
    for c in range(N_CORES):
        cols = slice(c * PER_CORE, (c + 1) * PER_CORE)
        a_pre_c = np.ascontiguousarray(T[c * M_TILES:(c + 1) * M_TILES])
        xt_c = np.ascontiguousarray(
            XT[:, cols].reshape(2, 128, PER_CORE).transpose(1, 0, 2))
        in_maps.append({
            "a_pre": a_pre_c,
            "xt": xt_c,
            "w1": w1_host,
            "w2": w2_host,
            "b1": b1_host,
            "b2": b2_host,
            "ident": ident,
        })
    return in_maps


def _get_nc(agg_mode=None):
    key = f"nc_{agg_mode or AGG_MODE}"
    if key not in _CACHE:
        _CACHE[key] = _build_nc(agg_mode or AGG_MODE)
    return _CACHE[key]


def kernel(X, A, W1, b1, W2, b2, _trace=False, _trace_kwargs=None):
    nc = _get_nc()
    in_maps = _prep_inputs(X, A, W1, b1, W2, b2, AGG_MODE)
    kw = {}
    if _trace:
        kw.update(trace=True, **(_trace_kwargs or {}))
    res = bass_utils.run_bass_kernel_spmd(
        nc, in_maps, core_ids=list(range(N_CORES)), **kw)
    _CACHE["last_result"] = res
    out = np.concatenate([res.results[c]["out"] for c in range(N_CORES)],
                         axis=0)[:N]
    return np.ascontiguousarray(out.astype(np.float32))


# revision 12
# speedup vs baseline: 1.1272x; 1.1272x over previous
"""Trainium2 Bass kernel for nn_Net_43052752175597 (2-layer GraphSAGE, aggr=add).

reference:
    A_hat = (A != 0).T with unit diagonal          # [N, N], binary
    h1   = X @ W1;  agg1 = A_hat @ h1 + b1;  x1 = relu(l2norm(agg1))
    h2   = x1 @ W2; agg2 = A_hat @ h2 + b2;  out = l2norm(l2norm(agg2))

Sharding: row-shard A_hat (output nodes) across 8 cores. Each core owns a
contiguous block of 1280 padded nodes (N padded 10000 -> 10240). Per layer the
core computes h for its own nodes in fp16, AllGathers the transformed
features in two chunks (so the collectives and the post-collective SBUF
loads overlap the aggregation matmuls), then aggregates its A_hat row-slice
(fp8 tiles, SBUF-resident across both layers) against the gathered features.

The gathered-feature SBUF buffer is double-buffered across layers so layer-2
chunks can land while layer-1 aggregation still reads the layer-1 features.
The global source-node (k) order is permuted chunk-major/rank-major to match
the chunked AllGather output layout; the host-side A_hat^T tiling applies the
same permutation, so on-device indexing stays trivial.

Precision: A is exactly 0/1 so fp8 A tiles are exact. h flows as fp16
(2^-11 rounding) with f32 PSUM accumulation -> ~1e-4 final rel err.
"""

import sys

sys.path.insert(0, "/opt/trn_rl_repo")

import numpy as np
import ml_dtypes

import concourse.bass as bass
import concourse.tile as tile
from concourse import bacc, mybir
from concourse import bass_utils

N = 10000
NP = 10240          # padded node count
F = 256             # input feature dim
H = 128             # hidden dim
N_CORES = 8
PER_CORE = NP // N_CORES        # 1280 nodes per core
M_TILES = PER_CORE // 128       # 10
K_TILES = NP // 128             # 80
# AllGather chunk geometry (shared by both layers; fixes the t_hfull slot
# layout). Asymmetric: a big chunk fired one m-tile early, then a tiny tail
# chunk so the seam between layers exposes only a minimal collective.
CHUNK_BOUNDS = [0, 5, 9, M_TILES]       # m-tile ranges per chunk
N_CHUNKS = len(CHUNK_BOUNDS) - 1
CHUNK_SIZES = [CHUNK_BOUNDS[i + 1] - CHUNK_BOUNDS[i] for i in range(N_CHUNKS)]
SLOT_BASE = [N_CORES * sum(CHUNK_SIZES[:i]) for i in range(N_CHUNKS)]

AGG_MODE = "fp16"

_CACHE = {}


def _build_nc(agg_mode=AGG_MODE, single_core=False, compile=True, repeats=1):
    """Build + compile the 8-core SPMD Bass kernel. Returns the Bacc object.

    single_core=True builds a 1-core variant with the collectives replaced by
    equivalent-byte local DMAs — only for TimelineSim cost-model profiling.
    repeats>1 runs the whole 2-layer body N times (benchmarking only).
    """
    fp32 = mybir.dt.float32
    fp16 = mybir.dt.float16
    fp8 = mybir.dt.float8e4

    nc = bacc.Bacc(
        "TRN2",
        target_bir_lowering=False,
        debug=False,
        enable_asserts=True,
        num_devices=1 if single_core else N_CORES,
    )

    # Per-core external inputs
    a_pre = nc.dram_tensor("a_pre", [M_TILES, 128, K_TILES, 128], fp8,
                           kind="ExternalInput").ap()
    xt = nc.dram_tensor("xt", [128, 2, PER_CORE], fp16,
                        kind="ExternalInput").ap()
    w1 = nc.dram_tensor("w1", [128, 2, H], fp16, kind="ExternalInput").ap()
    w2 = nc.dram_tensor("w2", [128, H], fp16, kind="ExternalInput").ap()
    b1 = nc.dram_tensor("b1", [128, H], fp32, kind="ExternalInput").ap()
    b2 = nc.dram_tensor("b2", [128, H], fp32, kind="ExternalInput").ap()
    ident = nc.dram_tensor("ident", [128, 128], fp32, kind="ExternalInput").ap()
    out = nc.dram_tensor("out", [PER_CORE, H], fp32, kind="ExternalOutput").ap()

    with tile.TileContext(nc) as tc:
        with tc.tile_pool(name="const", bufs=1) as cpool, \
             tc.tile_pool(name="hfull", bufs=2) as hpool, \
             tc.tile_pool(name="acol", bufs=1) as apool, \
             tc.tile_pool(name="work", bufs=1) as wpool, \
             tc.tile_pool(name="shard", bufs=2) as spool, \
             tc.tile_pool(name="psum_agg", bufs=4, space="PSUM") as pagg, \
             tc.tile_pool(name="psum_aux", bufs=2, space="PSUM") as paux, \
             tc.tile_pool(name="psum_tr", bufs=2, space="PSUM") as ptr, \
             tc.tile_pool(name="dram", bufs=2, space="DRAM") as dpool:

            # ---- constants into SBUF (gather-critical path first) ----
            t_xt = cpool.tile([128, 2, PER_CORE], fp16)
            t_w1 = cpool.tile([128, 2, H], fp16)
            t_w2 = cpool.tile([128, H], fp16)
            t_b1 = cpool.tile([128, H], fp32)
            t_b2 = cpool.tile([128, H], fp32)
            t_id = cpool.tile([128, 128], fp32)
            # split so the first h1 m-tiles can start before the whole X^T
            # slice has landed
            nc.sync.dma_start(t_xt[:, :, 0:PER_CORE // 2],
                              xt[:, :, 0:PER_CORE // 2])
            nc.sync.dma_start(t_xt[:, :, PER_CORE // 2:],
                              xt[:, :, PER_CORE // 2:])
            nc.sync.dma_start(t_w1[:], w1[:])
            nc.sync.dma_start(t_w2[:], w2[:])
            nc.sync.dma_start(t_b1[:], b1[:])
            nc.sync.dma_start(t_b2[:], b2[:])
            nc.sync.dma_start(t_id[:], ident[:])

            # Whole per-core A slice stays resident in SBUF, loaded once
            # (m-major so early m-tiles can start ASAP) and reused by both
            # layers. SWDGE so these bulk loads don't queue ahead of the
            # latency-critical HWDGE gather/feature DMAs.
            t_acache = apool.tile([128, M_TILES, K_TILES, 128], fp8)
            for m in range(M_TILES):
                nc.gpsimd.dma_start(t_acache[:, m, :, :], a_pre[m])

            # per-layer working tiles
            t_agg = wpool.tile([128, M_TILES, H], fp32, tag="agg")
            t_x1 = wpool.tile([128, M_TILES, H], fp32)
            t_x1t = wpool.tile([128, M_TILES, H], fp16)
            t_ssq = wpool.tile([128, M_TILES], fp32, tag="ssq")
            t_nrm = wpool.tile([128, M_TILES], fp32, tag="nrm")
            t_inv = wpool.tile([128, M_TILES], fp32, tag="inv")
            t_sq_scratch = wpool.tile([128, H], fp32)
            t_outf = wpool.tile([128, M_TILES, H], fp32)

            def gather_chunk(h_shard, ck, t_hfull, tag):
                """AllGather own m-tiles [CHUNK_BOUNDS[ck], CHUNK_BOUNDS[ck+1])
                of h_shard into t_hfull slots starting at SLOT_BASE[ck]
                (rank-major layout within the chunk)."""
                mc = CHUNK_SIZES[ck]
                kc = N_CORES * mc
                base = SLOT_BASE[ck]
                msl = slice(CHUNK_BOUNDS[ck], CHUNK_BOUNDS[ck + 1])
                sh_dram = dpool.tile([mc * 128, H], fp16, tag=f"sh_dram{ck}",
                                     name=f"sh_{tag}")
                g_dram = dpool.tile([kc * 128, H], fp16,
                                    tag=f"g_dram{ck}", name=f"g_{tag}",
                                    addr_space="Shared")
                nc.sync.dma_start(
                    sh_dram[:].rearrange("(m p) f -> p m f", p=128),
                    h_shard[:, msl, :])
                if single_core:
                    for r in range(N_CORES):
                        ks = base + r * mc
                        nc.sync.dma_start(
                            t_hfull[:, ks:ks + mc, :],
                            sh_dram[:].rearrange("(m p) f -> p m f", p=128))
                else:
                    nc.gpsimd.collective_compute(
                        "AllGather", mybir.AluOpType.bypass,
                        replica_groups=[list(range(N_CORES))],
                        ins=[sh_dram.opt()], outs=[g_dram.opt()],
                    )
                    # split the SBUF load so aggregation can start on the
                    # first ranks' k-tiles while the rest still loads
                    gv = g_dram[:].rearrange("(k p) f -> p k f", p=128)
                    kh = kc // 2 if kc > 16 else kc
                    nc.sync.dma_start(
                        t_hfull[:, base:base + kh, :], gv[:, 0:kh, :])
                    if kh < kc:
                        nc.sync.dma_start(
                            t_hfull[:, base + kh:base + kc, :],
                            gv[:, kh:kc, :])

            def aggregate(m, t_bias, t_hfull):
                """t_agg[:, m, :] = A_hat-slice @ h + bias, plus row sum-sq."""
                ps = pagg.tile([128, H], fp32, tag="ps_agg")
                for k in range(K_TILES):
                    nc.tensor.matmul(ps[:], t_acache[:, m, k, :],
                                     t_hfull[:, k, :],
                                     start=(k == 0), stop=(k == K_TILES - 1))
                nc.vector.tensor_tensor(t_agg[:, m, :], ps[:, 0:H], t_bias[:],
                                        op=mybir.AluOpType.add)
                nc.scalar.activation(t_sq_scratch[:], t_agg[:, m, :],
                                     mybir.ActivationFunctionType.Square,
                                     accum_out=t_ssq[:, m:m + 1])

            for _rep in range(repeats):
              t_h1full = hpool.tile([128, K_TILES, H], fp16, tag="hfull",
                                    name=f"h1full_{_rep}")
              t_h2full = hpool.tile([128, K_TILES, H], fp16, tag="hfull",
                                    name=f"h2full_{_rep}")

              # =============== Layer 1: h1 = fp16(X @ W1) for own nodes ====
              h1_shard = spool.tile([128, M_TILES, H], fp16, tag="h_shard",
                                    name=f"h1_shard_{_rep}")
              for m in range(M_TILES):
                ps = paux.tile([128, H], fp32, tag="ps_h")
                for k in range(2):
                    nc.tensor.matmul(ps[:],
                                     t_xt[:, k, m * 128:(m + 1) * 128],
                                     t_w1[:, k, :],
                                     start=(k == 0), stop=(k == 1))
                nc.vector.tensor_copy(h1_shard[:, m, :], ps[:])
                for ck in range(N_CHUNKS):
                    if m == CHUNK_BOUNDS[ck + 1] - 1:
                        gather_chunk(h1_shard, ck, t_h1full,
                                     f"h1c{ck}_{_rep}")

              # ====== Layer 1 aggregation fused with norm/relu/h2 per m-tile.
              # The PE-side tail (transpose + h2 matmul) is software-pipelined
              # one m-tile behind the aggregation so the PE never waits on the
              # serial norm chain.
              h2_shard = spool.tile([128, M_TILES, H], fp16, tag="h_shard",
                                    name=f"h2_shard_{_rep}")

              def pe_tail(m):
                """transpose x1 tile -> fp16 stationary; h2 = x1t^T @ W2."""
                pst = ptr.tile([128, 128], fp32, tag="pst")
                nc.tensor.transpose(pst[:], t_x1[:, m, :], t_id[:])
                nc.vector.tensor_copy(t_x1t[:, m, :], pst[:])
                ps2 = paux.tile([128, H], fp32, tag="ps_h")
                nc.tensor.matmul(ps2[:], t_x1t[:, m, :], t_w2[:],
                                 start=True, stop=True)
                nc.vector.tensor_copy(h2_shard[:, m, :], ps2[:])
                for ck in range(N_CHUNKS):
                    if m == CHUNK_BOUNDS[ck + 1] - 1:
                        gather_chunk(h2_shard, ck, t_h2full,
                                     f"h2c{ck}_{_rep}")

              for m in range(M_TILES):
                aggregate(m, t_b1, t_h1full)
                msl = slice(m, m + 1)
                # ||agg|| >= ~||b|| >> 1e-12 always, so skip the eps clamp
                nc.scalar.sqrt(t_nrm[:, msl], t_ssq[:, msl])
                nc.vector.reciprocal(t_inv[:, msl], t_nrm[:, msl])
                # x1 = relu(agg1 / max(||agg1||, 1e-12))
                nc.scalar.activation(t_x1[:, m, :], t_agg[:, m, :],
                                     mybir.ActivationFunctionType.Relu,
                                     scale=t_inv[:, msl])
                if m >= 1:
                    pe_tail(m - 1)
              pe_tail(M_TILES - 1)

              # ====== Layer 2 aggregation fused with the final l2norm.
              # l2norm(l2norm(x)) == l2norm(x) up to f32 rounding whenever
              # ||x|| > eps (always: agg2 includes the b2 offset), since the
              # inner normalize yields a unit-norm vector.
              for m in range(M_TILES):
                aggregate(m, t_b2, t_h2full)
                msl = slice(m, m + 1)
                nc.scalar.sqrt(t_nrm[:, msl], t_ssq[:, msl])
                nc.vector.reciprocal(t_inv[:, msl], t_nrm[:, msl])
                nc.scalar.activation(t_outf[:, m, :], t_agg[:, m, :],
                                     mybir.ActivationFunctionType.Copy,
                                     scale=t_inv[:, msl])
                nc.sync.dma_start(
                    out[:].rearrange("(mm p) f -> p mm f", p=128)[:, m, :],
                    t_outf[:, m, :])

    if compile:
        nc.compile()
    return nc


def _k_perm():
    """New k-tile order: chunk-major, rank-major, tile-minor.
    perm[new_k] = old_k = rank*M_TILES + (chunk m-range start) + t."""
    perm = np.empty(K_TILES, dtype=np.int64)
    for ck in range(N_CHUNKS):
        mc = CHUNK_SIZES[ck]
        for r in range(N_CORES):
            for t in range(mc):
                perm[SLOT_BASE[ck] + r * mc + t] = \
                    r * M_TILES + CHUNK_BOUNDS[ck] + t
    return perm


def _prep_inputs(X, A, W1, b1, W2, b2, agg_mode=AGG_MODE):
    """Host-side sharding/layout prep. Returns in_maps for the 8 cores."""
    f32 = np.float32
    fp16 = np.float16
    a_np_dt = ml_dtypes.float8_e4m3

    # --- A_hat^T = binarized A with unit diagonal, padded, tiled ---
    Ab = np.zeros((NP, NP), dtype=a_np_dt)
    Ab[:N, :N] = (A != 0)
    idx = np.arange(N)
    Ab[idx, idx] = 1.0
    # [k, p, cm, n] -> [cm, p, k, n], then permute k to the chunked layout
    T = Ab.reshape(K_TILES, 128, K_TILES, 128).transpose(2, 1, 0, 3)
    T = T[:, :, _k_perm(), :]

    # --- X^T fp16, padded ---
    Xp = np.zeros((NP, F), dtype=f32)
    Xp[:N] = np.asarray(X, dtype=f32)
    XT = np.ascontiguousarray(Xp.T).astype(fp16)     # [256, NP]

    w1_host = np.ascontiguousarray(
        np.asarray(W1, dtype=f32).reshape(2, 128, H).transpose(1, 0, 2)
    ).astype(fp16)                                   # [128, 2, H]
    w2_host = np.asarray(W2, dtype=f32).astype(fp16)  # [128, H]

    b1_host = np.ascontiguousarray(
        np.broadcast_to(np.asarray(b1, dtype=f32), (128, H)))
    b2_host = np.ascontiguousarray(
        np.broadcast_to(np.asarray(b2, dtype=f32), (128, H)))
    ident = np.eye(128, dtype=f32)

    in_maps = []
    for c in range(N_CORES):
        cols = slice(c * PER_CORE, (c + 1) * PER_CORE)
        a_pre_c = np.ascontiguousarray(T[c * M_TILES:(c + 1) * M_TILES])
        xt_c = np.ascontiguousarray(
            XT[:, cols].reshape(2, 128, PER_CORE).transpose(1, 0, 2))
        in_maps.append({
            "a_pre": a_pre_c,
            "xt": xt_c,
            "w1": w1_host,
            "w2": w2_host,
            "b1": b1_host,
            "b2": b2_host,
            "ident": ident,
        })
    return in_maps


def _get_nc(agg_mode=None):
    key = f"nc_{agg_mode or AGG_MODE}"
    if key not in _CACHE:
        _CACHE[key] = _build_nc(agg_mode or AGG_MODE)
    return _CACHE[key]


def kernel(X, A, W1, b1, W2, b2, _trace=False, _trace_kwargs=None):
    nc = _get_nc()
    in_maps = _prep_inputs(X, A, W1, b1, W2, b2, AGG_MODE)
    kw = {}
    if _trace:
        kw.update(trace=True, **(_trace_kwargs or {}))
    res = bass_utils.run_bass_kernel_spmd(
        nc, in_maps, core_ids=list(range(N_CORES)), **kw)
    _CACHE["last_result"] = res
    out = np.concatenate([res.results[c]["out"] for c in range(N_CORES)],
                         axis=0)[:N]
    return np.ascontiguousarray(out.astype(np.float32))


# revision 16
# speedup vs baseline: 1.6174x; 1.4348x over previous
"""Trainium2 Bass kernel for nn_Net_43052752175597 (2-layer GraphSAGE, aggr=add).

reference:
    A_hat = (A != 0).T with unit diagonal          # [N, N], binary
    h1   = X @ W1;  agg1 = A_hat @ h1 + b1;  x1 = relu(l2norm(agg1))
    h2   = x1 @ W2; agg2 = A_hat @ h2 + b2;  out = l2norm(l2norm(agg2))

Sharding: row-shard A_hat (output nodes) across 8 cores. Each core owns a
contiguous block of 1280 padded nodes (N padded 10000 -> 10240). Per layer the
core computes h for its own nodes in fp16, AllGathers the transformed
features in two chunks (so the collectives and the post-collective SBUF
loads overlap the aggregation matmuls), then aggregates its A_hat row-slice
(fp8 tiles, SBUF-resident across both layers) against the gathered features.

The gathered-feature SBUF buffer is double-buffered across layers so layer-2
chunks can land while layer-1 aggregation still reads the layer-1 features.
The global source-node (k) order is permuted chunk-major/rank-major to match
the chunked AllGather output layout; the host-side A_hat^T tiling applies the
same permutation, so on-device indexing stays trivial.

Precision: A is exactly 0/1 so fp8 A tiles are exact. h flows as fp16
(2^-11 rounding) with f32 PSUM accumulation -> ~1e-4 final rel err.
"""

import sys

sys.path.insert(0, "/opt/trn_rl_repo")

import numpy as np
import ml_dtypes

import concourse.bass as bass
import concourse.tile as tile
from concourse import bacc, mybir
from concourse import bass_utils

N = 10000
NP = 10240          # padded node count
F = 256             # input feature dim
H = 128             # hidden dim
N_CORES = 8
PER_CORE = NP // N_CORES        # 1280 nodes per core
M_TILES = PER_CORE // 128       # 10
K_TILES = NP // 128             # 80
# AllGather chunk geometry (shared by both layers; fixes the t_hfull slot
# layout). Asymmetric: a big chunk fired one m-tile early, then a tiny tail
# chunk so the seam between layers exposes only a minimal collective.
CHUNK_BOUNDS = [0, 5, 9, M_TILES]       # m-tile ranges per chunk
N_CHUNKS = len(CHUNK_BOUNDS) - 1
CHUNK_SIZES = [CHUNK_BOUNDS[i + 1] - CHUNK_BOUNDS[i] for i in range(N_CHUNKS)]
SLOT_BASE = [N_CORES * sum(CHUNK_SIZES[:i]) for i in range(N_CHUNKS)]

AGG_MODE = "fp16"

# DoubleRow (fp8) variant: chunk geometry aligned to the PSUM column groups
CHUNK_BOUNDS_DR = [0, 4, 8, M_TILES]
GROUPS_DR = [(0, 512), (512, 1024), (1024, PER_CORE)]
KP = K_TILES // 2               # 40 k-tile pairs

_CACHE = {}


def _build_nc(agg_mode=AGG_MODE, single_core=False, compile=True, repeats=1):
    """Build + compile the 8-core SPMD Bass kernel. Returns the Bacc object.

    single_core=True builds a 1-core variant with the collectives replaced by
    equivalent-byte local DMAs — only for TimelineSim cost-model profiling.
    repeats>1 runs the whole 2-layer body N times (benchmarking only).
    """
    if agg_mode == "fp8dr":
        return _build_nc_dr(single_core=single_core, compile=compile,
                            repeats=repeats)
    fp32 = mybir.dt.float32
    fp16 = mybir.dt.float16
    fp8 = mybir.dt.float8e4

    nc = bacc.Bacc(
        "TRN2",
        target_bir_lowering=False,
        debug=False,
        enable_asserts=True,
        num_devices=1 if single_core else N_CORES,
    )

    # Per-core external inputs
    a_pre = nc.dram_tensor("a_pre", [M_TILES, 128, K_TILES, 128], fp8,
                           kind="ExternalInput").ap()
    xt = nc.dram_tensor("xt", [128, 2, PER_CORE], fp16,
                        kind="ExternalInput").ap()
    w1 = nc.dram_tensor("w1", [128, 2, H], fp16, kind="ExternalInput").ap()
    w2 = nc.dram_tensor("w2", [128, H], fp16, kind="ExternalInput").ap()
    b1 = nc.dram_tensor("b1", [128, H], fp32, kind="ExternalInput").ap()
    b2 = nc.dram_tensor("b2", [128, H], fp32, kind="ExternalInput").ap()
    ident = nc.dram_tensor("ident", [128, 128], fp32, kind="ExternalInput").ap()
    out = nc.dram_tensor("out", [PER_CORE, H], fp32, kind="ExternalOutput").ap()

    with tile.TileContext(nc) as tc:
        with tc.tile_pool(name="const", bufs=1) as cpool, \
             tc.tile_pool(name="hfull", bufs=2) as hpool, \
             tc.tile_pool(name="acol", bufs=1) as apool, \
             tc.tile_pool(name="work", bufs=1) as wpool, \
             tc.tile_pool(name="shard", bufs=2) as spool, \
             tc.tile_pool(name="psum_agg", bufs=4, space="PSUM") as pagg, \
             tc.tile_pool(name="psum_aux", bufs=2, space="PSUM") as paux, \
             tc.tile_pool(name="psum_tr", bufs=2, space="PSUM") as ptr, \
             tc.tile_pool(name="dram", bufs=2, space="DRAM") as dpool:

            # ---- constants into SBUF (gather-critical path first) ----
            t_xt = cpool.tile([128, 2, PER_CORE], fp16)
            t_w1 = cpool.tile([128, 2, H], fp16)
            t_w2 = cpool.tile([128, H], fp16)
            t_b1 = cpool.tile([128, H], fp32)
            t_b2 = cpool.tile([128, H], fp32)
            t_id = cpool.tile([128, 128], fp32)
            # split so the first h1 m-tiles can start before the whole X^T
            # slice has landed
            nc.sync.dma_start(t_xt[:, :, 0:PER_CORE // 2],
                              xt[:, :, 0:PER_CORE // 2])
            nc.sync.dma_start(t_xt[:, :, PER_CORE // 2:],
                              xt[:, :, PER_CORE // 2:])
            nc.sync.dma_start(t_w1[:], w1[:])
            nc.sync.dma_start(t_w2[:], w2[:])
            nc.sync.dma_start(t_b1[:], b1[:])
            nc.sync.dma_start(t_b2[:], b2[:])
            nc.sync.dma_start(t_id[:], ident[:])

            # Whole per-core A slice stays resident in SBUF, loaded once
            # (m-major so early m-tiles can start ASAP) and reused by both
            # layers. SWDGE so these bulk loads don't queue ahead of the
            # latency-critical HWDGE gather/feature DMAs.
            t_acache = apool.tile([128, M_TILES, K_TILES, 128], fp8)
            for m in range(M_TILES):
                nc.gpsimd.dma_start(t_acache[:, m, :, :], a_pre[m])

            # per-layer working tiles
            t_agg = wpool.tile([128, M_TILES, H], fp32, tag="agg")
            t_x1 = wpool.tile([128, M_TILES, H], fp32)
            t_x1t = wpool.tile([128, M_TILES, H], fp16)
            t_ssq = wpool.tile([128, M_TILES], fp32, tag="ssq")
            t_nrm = wpool.tile([128, M_TILES], fp32, tag="nrm")
            t_inv = wpool.tile([128, M_TILES], fp32, tag="inv")
            t_sq_scratch = wpool.tile([128, H], fp32)
            t_outf = wpool.tile([128, M_TILES, H], fp32)

            def gather_chunk(h_shard, ck, t_hfull, tag):
                """AllGather own m-tiles [CHUNK_BOUNDS[ck], CHUNK_BOUNDS[ck+1])
                of h_shard into t_hfull slots starting at SLOT_BASE[ck]
                (rank-major layout within the chunk)."""
                mc = CHUNK_SIZES[ck]
                kc = N_CORES * mc
                base = SLOT_BASE[ck]
                msl = slice(CHUNK_BOUNDS[ck], CHUNK_BOUNDS[ck + 1])
                sh_dram = dpool.tile([mc * 128, H], fp16, tag=f"sh_dram{ck}",
                                     name=f"sh_{tag}")
                g_dram = dpool.tile([kc * 128, H], fp16,
                                    tag=f"g_dram{ck}", name=f"g_{tag}",
                                    addr_space="Shared")
                nc.sync.dma_start(
                    sh_dram[:].rearrange("(m p) f -> p m f", p=128),
                    h_shard[:, msl, :])
                if single_core:
                    for r in range(N_CORES):
                        ks = base + r * mc
                        nc.sync.dma_start(
                            t_hfull[:, ks:ks + mc, :],
                            sh_dram[:].rearrange("(m p) f -> p m f", p=128))
                else:
                    nc.gpsimd.collective_compute(
                        "AllGather", mybir.AluOpType.bypass,
                        replica_groups=[list(range(N_CORES))],
                        ins=[sh_dram.opt()], outs=[g_dram.opt()],
                    )
                    # split the SBUF load so aggregation can start on the
                    # first ranks' k-tiles while the rest still loads
                    gv = g_dram[:].rearrange("(k p) f -> p k f", p=128)
                    kh = kc // 2 if kc > 16 else kc
                    nc.sync.dma_start(
                        t_hfull[:, base:base + kh, :], gv[:, 0:kh, :])
                    if kh < kc:
                        nc.sync.dma_start(
                            t_hfull[:, base + kh:base + kc, :],
                            gv[:, kh:kc, :])

            def aggregate(m, t_bias, t_hfull):
                """t_agg[:, m, :] = A_hat-slice @ h + bias, plus row sum-sq."""
                ps = pagg.tile([128, H], fp32, tag="ps_agg")
                for k in range(K_TILES):
                    nc.tensor.matmul(ps[:], t_acache[:, m, k, :],
                                     t_hfull[:, k, :],
                                     start=(k == 0), stop=(k == K_TILES - 1))
                nc.vector.tensor_tensor(t_agg[:, m, :], ps[:, 0:H], t_bias[:],
                                        op=mybir.AluOpType.add)
                nc.scalar.activation(t_sq_scratch[:], t_agg[:, m, :],
                                     mybir.ActivationFunctionType.Square,
                                     accum_out=t_ssq[:, m:m + 1])

            for _rep in range(repeats):
              t_h1full = hpool.tile([128, K_TILES, H], fp16, tag="hfull",
                                    name=f"h1full_{_rep}")
              t_h2full = hpool.tile([128, K_TILES, H], fp16, tag="hfull",
                                    name=f"h2full_{_rep}")

              # =============== Layer 1: h1 = fp16(X @ W1) for own nodes ====
              h1_shard = spool.tile([128, M_TILES, H], fp16, tag="h_shard",
                                    name=f"h1_shard_{_rep}")
              for m in range(M_TILES):
                ps = paux.tile([128, H], fp32, tag="ps_h")
                for k in range(2):
                    nc.tensor.matmul(ps[:],
                                     t_xt[:, k, m * 128:(m + 1) * 128],
                                     t_w1[:, k, :],
                                     start=(k == 0), stop=(k == 1))
                nc.vector.tensor_copy(h1_shard[:, m, :], ps[:])
                for ck in range(N_CHUNKS):
                    if m == CHUNK_BOUNDS[ck + 1] - 1:
                        gather_chunk(h1_shard, ck, t_h1full,
                                     f"h1c{ck}_{_rep}")

              # ====== Layer 1 aggregation fused with norm/relu/h2 per m-tile.
              # The PE-side tail (transpose + h2 matmul) is software-pipelined
              # one m-tile behind the aggregation so the PE never waits on the
              # serial norm chain.
              h2_shard = spool.tile([128, M_TILES, H], fp16, tag="h_shard",
                                    name=f"h2_shard_{_rep}")

              def pe_tail(m):
                """transpose x1 tile -> fp16 stationary; h2 = x1t^T @ W2."""
                pst = ptr.tile([128, 128], fp32, tag="pst")
                nc.tensor.transpose(pst[:], t_x1[:, m, :], t_id[:])
                nc.vector.tensor_copy(t_x1t[:, m, :], pst[:])
                ps2 = paux.tile([128, H], fp32, tag="ps_h")
                nc.tensor.matmul(ps2[:], t_x1t[:, m, :], t_w2[:],
                                 start=True, stop=True)
                nc.vector.tensor_copy(h2_shard[:, m, :], ps2[:])
                for ck in range(N_CHUNKS):
                    if m == CHUNK_BOUNDS[ck + 1] - 1:
                        gather_chunk(h2_shard, ck, t_h2full,
                                     f"h2c{ck}_{_rep}")

              for m in range(M_TILES):
                aggregate(m, t_b1, t_h1full)
                msl = slice(m, m + 1)
                # ||agg|| >= ~||b|| >> 1e-12 always, so skip the eps clamp
                nc.scalar.sqrt(t_nrm[:, msl], t_ssq[:, msl])
                nc.vector.reciprocal(t_inv[:, msl], t_nrm[:, msl])
                # x1 = relu(agg1 / max(||agg1||, 1e-12))
                nc.scalar.activation(t_x1[:, m, :], t_agg[:, m, :],
                                     mybir.ActivationFunctionType.Relu,
                                     scale=t_inv[:, msl])
                if m >= 1:
                    pe_tail(m - 1)
              pe_tail(M_TILES - 1)

              # ====== Layer 2 aggregation fused with the final l2norm.
              # l2norm(l2norm(x)) == l2norm(x) up to f32 rounding whenever
              # ||x|| > eps (always: agg2 includes the b2 offset), since the
              # inner normalize yields a unit-norm vector.
              for m in range(M_TILES):
                aggregate(m, t_b2, t_h2full)
                msl = slice(m, m + 1)
                nc.scalar.sqrt(t_nrm[:, msl], t_ssq[:, msl])
                nc.vector.reciprocal(t_inv[:, msl], t_nrm[:, msl])
                nc.scalar.activation(t_outf[:, m, :], t_agg[:, m, :],
                                     mybir.ActivationFunctionType.Copy,
                                     scale=t_inv[:, msl])
                nc.sync.dma_start(
                    out[:].rearrange("(mm p) f -> p mm f", p=128)[:, m, :],
                    t_outf[:, m, :])

    if compile:
        nc.compile()
    return nc


def _k_perm(bounds=None):
    """New k-tile order: chunk-major, rank-major, tile-minor.
    perm[new_k] = old_k = rank*M_TILES + (chunk m-range start) + t."""
    bounds = bounds or CHUNK_BOUNDS
    sizes = [bounds[i + 1] - bounds[i] for i in range(len(bounds) - 1)]
    bases = [N_CORES * sum(sizes[:i]) for i in range(len(sizes))]
    perm = np.empty(K_TILES, dtype=np.int64)
    for ck in range(len(sizes)):
        for r in range(N_CORES):
            for t in range(sizes[ck]):
                perm[bases[ck] + r * sizes[ck] + t] = \
                    r * M_TILES + bounds[ck] + t
    return perm


def _build_nc_dr(single_core=False, compile=True, repeats=1):
    """DoubleRow fp8 variant: h-stationary aggregation in feature-major
    layout. Stationary = gathered h (fp8, 2 k-tiles packed per LDWEIGHTS),
    moving = A_hat^T column slices (fp8, exact binary), PSUM accumulates
    agg^T = [128 feat, own nodes] across 3 column groups interleaved per
    k-pair so one weight load feeds 3 wide DoubleRow matmuls (~2x the
    bf16-rate aggregation throughput).

    Norms: per-node sum-of-squares via tiny ones-column matmuls into a
    [10, 128] PSUM tile, then one PE transpose redistributes 1/norm to
    node-major. h2 = relu(agg1^T)-tiles (fp16 stationary) @ W2 with the
    1/norm folded in afterwards (scale is linear). Final output leaves
    feature-major via 10 PE transposes.
    """
    fp32 = mybir.dt.float32
    fp16 = mybir.dt.float16
    fp8 = mybir.dt.float8e4
    DR = mybir.MatmulPerfMode.DoubleRow
    bounds = CHUNK_BOUNDS_DR
    n_chunks = len(bounds) - 1
    sizes = [bounds[i + 1] - bounds[i] for i in range(n_chunks)]
    bases = [N_CORES * sum(sizes[:i]) for i in range(n_chunks)]

    nc = bacc.Bacc(
        "TRN2",
        target_bir_lowering=False,
        debug=False,
        enable_asserts=True,
        num_devices=1 if single_core else N_CORES,
    )

    a_t = nc.dram_tensor("a_t", [128, K_TILES, PER_CORE], fp8,
                         kind="ExternalInput").ap()
    xt = nc.dram_tensor("xt", [128, 2, PER_CORE], fp16,
                        kind="ExternalInput").ap()
    w1 = nc.dram_tensor("w1", [128, 2, H], fp16, kind="ExternalInput").ap()
    w2 = nc.dram_tensor("w2", [128, H], fp16, kind="ExternalInput").ap()
    b1 = nc.dram_tensor("b1", [128, 1], fp32, kind="ExternalInput").ap()
    b2 = nc.dram_tensor("b2", [128, 1], fp32, kind="ExternalInput").ap()
    ones10 = nc.dram_tensor("ones10", [128, M_TILES, M_TILES], fp16,
                            kind="ExternalInput").ap()
    ident = nc.dram_tensor("ident", [128, 128], fp32, kind="ExternalInput").ap()
    out = nc.dram_tensor("out", [PER_CORE, H], fp32, kind="ExternalOutput").ap()

    with tile.TileContext(nc) as tc:
        with tc.tile_pool(name="const", bufs=1) as cpool, \
             tc.tile_pool(name="hfull", bufs=2) as hpool, \
             tc.tile_pool(name="acol", bufs=1) as apool, \
             tc.tile_pool(name="work", bufs=1) as wpool, \
             tc.tile_pool(name="shard", bufs=2) as spool, \
             tc.tile_pool(name="psum_agg", bufs=1, space="PSUM") as pagg, \
             tc.tile_pool(name="psum_scr", bufs=2, space="PSUM") as pscr, \
             tc.tile_pool(name="psum_nrm", bufs=1, space="PSUM") as pnrm, \
             tc.tile_pool(name="dram", bufs=2, space="DRAM") as dpool:

            t_xt = cpool.tile([128, 2, PER_CORE], fp16)
            t_w1 = cpool.tile([128, 2, H], fp16)
            t_w2 = cpool.tile([128, H], fp16)
            t_b1 = cpool.tile([128, 1], fp32)
            t_b2 = cpool.tile([128, 1], fp32)
            t_e10 = cpool.tile([128, M_TILES, M_TILES], fp16)
            t_id = cpool.tile([128, 128], fp32)
            nc.sync.dma_start(t_xt[:, :, 0:PER_CORE // 2],
                              xt[:, :, 0:PER_CORE // 2])
            nc.sync.dma_start(t_xt[:, :, PER_CORE // 2:],
                              xt[:, :, PER_CORE // 2:])
            nc.sync.dma_start(t_w1[:], w1[:])
            nc.sync.dma_start(t_w2[:], w2[:])
            nc.sync.dma_start(t_b1[:], b1[:])
            nc.sync.dma_start(t_b2[:], b2[:])
            nc.sync.dma_start(t_e10[:], ones10[:])
            nc.sync.dma_start(t_id[:], ident[:])

            # A^T slices, k-major so the layer-1 k-loop can start early
            t_at = apool.tile([128, K_TILES, PER_CORE], fp8)
            for kq in range(20):
                nc.gpsimd.dma_start(t_at[:, 4 * kq:4 * kq + 4, :],
                                    a_t[:, 4 * kq:4 * kq + 4, :])

            t_aggT = wpool.tile([128, PER_CORE], fp32, tag="aggT")
            t_sq = wpool.tile([128, PER_CORE], fp16, tag="sq")
            t_x1T = wpool.tile([128, PER_CORE], fp16, tag="x1T")
            t_inv10 = wpool.tile([128, 128], fp32, tag="inv10")
            t_invnm = wpool.tile([128, M_TILES], fp32, tag="invnm")
            t_nrm10 = wpool.tile([10, 128], fp32, tag="nrm10")
            t_outf = wpool.tile([128, M_TILES, H], fp32)
            nc.vector.memset(t_inv10[:], 0.0)

            def gather_chunk(h_shard, ck, t_hfull, tag):
                mc = sizes[ck]
                kc = N_CORES * mc
                base = bases[ck]
                msl = slice(bounds[ck], bounds[ck + 1])
                sh_dram = dpool.tile([mc * 128, H], fp8, tag=f"sh_dram{ck}",
                                     name=f"sh_{tag}")
                g_dram = dpool.tile([kc * 128, H], fp8,
                                    tag=f"g_dram{ck}", name=f"g_{tag}",
                                    addr_space="Shared")
                nc.sync.dma_start(
                    sh_dram[:].rearrange("(m p) f -> p m f", p=128),
                    h_shard[:, msl, :])
                if single_core:
                    for r in range(N_CORES):
                        ks = base + r * mc
                        nc.sync.dma_start(
                            t_hfull[:, ks:ks + mc, :],
                            sh_dram[:].rearrange("(m p) f -> p m f", p=128))
                else:
                    nc.gpsimd.collective_compute(
                        "AllGather", mybir.AluOpType.bypass,
                        replica_groups=[list(range(N_CORES))],
                        ins=[sh_dram.opt()], outs=[g_dram.opt()],
                    )
                    gv = g_dram[:].rearrange("(k p) f -> p k f", p=128)
                    kh = kc // 2 if kc > 16 else kc
                    nc.sync.dma_start(
                        t_hfull[:, base:base + kh, :], gv[:, 0:kh, :])
                    if kh < kc:
                        nc.sync.dma_start(
                            t_hfull[:, base + kh:base + kc, :],
                            gv[:, kh:kc, :])

            def agg_T(t_hfull, rep_tag):
                """agg^T[feat, own nodes] in 3 PSUM groups, DoubleRow."""
                pss = [pagg.tile([128, g1 - g0], fp32, tag=f"agg{i}",
                                 name=f"agg{i}_{rep_tag}")
                       for i, (g0, g1) in enumerate(GROUPS_DR)]
                for kp in range(KP):
                    lhs = t_hfull[:, 2 * kp:2 * kp + 2, :]
                    for i, (g0, g1) in enumerate(GROUPS_DR):
                        nc.tensor.matmul(pss[i][:], lhs,
                                         t_at[:, 2 * kp:2 * kp + 2, g0:g1],
                                         start=(kp == 0), stop=(kp == KP - 1),
                                         perf_mode=DR)
                return pss

            def norms_to_invnm(rep_tag):
                """t_sq [feat, nodes] -> per-node 1/norm in node-major
                t_invnm[node, subtile]."""
                psn = pnrm.tile([10, 128], fp32, tag="psn",
                                name=f"psn_{rep_tag}")
                for j in range(M_TILES):
                    nc.tensor.matmul(psn[:], t_e10[:, j, :],
                                     t_sq[:, j * 128:(j + 1) * 128],
                                     start=(j == 0), stop=(j == M_TILES - 1))
                nc.scalar.sqrt(t_nrm10[:], psn[:])
                nc.vector.reciprocal(t_inv10[0:10, :], t_nrm10[:])
                pst = pscr.tile([128, 128], fp32, tag="scr",
                                name=f"invT_{rep_tag}")
                nc.tensor.transpose(pst[:], t_inv10[:], t_id[:])
                nc.vector.tensor_copy(t_invnm[:], pst[:, 0:M_TILES])

            for _rep in range(repeats):
              t_h1full = hpool.tile([128, K_TILES, H], fp8, tag="hfull",
                                    name=f"h1full_{_rep}")
              t_h2full = hpool.tile([128, K_TILES, H], fp8, tag="hfull",
                                    name=f"h2full_{_rep}")

              # ---- h1 = fp16(X @ W1), own nodes, to fp8 shard ----
              h1_shard = spool.tile([128, M_TILES, H], fp8, tag="h_shard",
                                    name=f"h1_shard_{_rep}")
              for j in range(M_TILES):
                ps = pscr.tile([128, H], fp32, tag="scr",
                               name=f"h1_{j}_{_rep}")
                for k in range(2):
                    nc.tensor.matmul(ps[:],
                                     t_xt[:, k, j * 128:(j + 1) * 128],
                                     t_w1[:, k, :],
                                     start=(k == 0), stop=(k == 1))
                nc.vector.tensor_copy(h1_shard[:, j, :], ps[:])
                for ck in range(n_chunks):
                    if j == bounds[ck + 1] - 1:
                        gather_chunk(h1_shard, ck, t_h1full,
                                     f"h1c{ck}_{_rep}")

              # ---- layer 1 aggregation + postprocess ----
              pss = agg_T(t_h1full, f"l1_{_rep}")
              for i, (g0, g1) in enumerate(GROUPS_DR):
                  nc.vector.tensor_scalar_add(t_aggT[:, g0:g1], pss[i][:],
                                              t_b1[:])
                  nc.scalar.activation(t_sq[:, g0:g1], t_aggT[:, g0:g1],
                                       mybir.ActivationFunctionType.Square)
                  nc.vector.tensor_scalar_max(t_x1T[:, g0:g1],
                                              t_aggT[:, g0:g1], 0.0)
              norms_to_invnm(f"l1_{_rep}")

              # ---- h2 = (relu(agg1^T) @ W2) * inv, own nodes, fp8 ----
              h2_shard = spool.tile([128, M_TILES, H], fp8, tag="h_shard",
                                    name=f"h2_shard_{_rep}")
              for j in range(M_TILES):
                ps2 = pscr.tile([128, H], fp32, tag="scr",
                                name=f"h2_{j}_{_rep}")
                nc.tensor.matmul(ps2[:], t_x1T[:, j * 128:(j + 1) * 128],
                                 t_w2[:], start=True, stop=True)
                nc.scalar.activation(h2_shard[:, j, :], ps2[:],
                                     mybir.ActivationFunctionType.Copy,
                                     scale=t_invnm[:, j:j + 1])
                for ck in range(n_chunks):
                    if j == bounds[ck + 1] - 1:
                        gather_chunk(h2_shard, ck, t_h2full,
                                     f"h2c{ck}_{_rep}")

              # ---- layer 2 aggregation + final l2norm + output ----
              pss2 = agg_T(t_h2full, f"l2_{_rep}")
              for i, (g0, g1) in enumerate(GROUPS_DR):
                  nc.vector.tensor_scalar_add(t_aggT[:, g0:g1], pss2[i][:],
                                              t_b2[:])
                  nc.scalar.activation(t_sq[:, g0:g1], t_aggT[:, g0:g1],
                                       mybir.ActivationFunctionType.Square)
              norms_to_invnm(f"l2_{_rep}")
              for j in range(M_TILES):
                pst = pscr.tile([128, 128], fp32, tag="scr",
                                name=f"outT_{j}_{_rep}")
                nc.tensor.transpose(pst[:], t_aggT[:, j * 128:(j + 1) * 128],
                                    t_id[:])
                nc.scalar.activation(t_outf[:, j, :], pst[:],
                                     mybir.ActivationFunctionType.Copy,
                                     scale=t_invnm[:, j:j + 1])
                nc.sync.dma_start(
                    out[:].rearrange("(mm p) f -> p mm f", p=128)[:, j, :],
                    t_outf[:, j, :])

    if compile:
        nc.compile()
    return nc


def _prep_inputs(X, A, W1, b1, W2, b2, agg_mode=AGG_MODE):
    """Host-side sharding/layout prep. Returns in_maps for the 8 cores."""
    f32 = np.float32
    fp16 = np.float16
    a_np_dt = ml_dtypes.float8_e4m3
    dr = agg_mode == "fp8dr"

    # --- A_hat^T = binarized A with unit diagonal, padded ---
    Ab = np.zeros((NP, NP), dtype=a_np_dt)
    Ab[:N, :N] = (A != 0)
    idx = np.arange(N)
    Ab[idx, idx] = 1.0

    # --- X^T fp16, padded ---
    Xp = np.zeros((NP, F), dtype=f32)
    Xp[:N] = np.asarray(X, dtype=f32)
    XT = np.ascontiguousarray(Xp.T).astype(fp16)     # [256, NP]

    w1_host = np.ascontiguousarray(
        np.asarray(W1, dtype=f32).reshape(2, 128, H).transpose(1, 0, 2)
    ).astype(fp16)                                   # [128, 2, H]
    w2_host = np.asarray(W2, dtype=f32).astype(fp16)  # [128, H]
    ident = np.eye(128, dtype=f32)

    if dr:
        perm = _k_perm(CHUNK_BOUNDS_DR)
        # a_t[p, k, n] = A_hat^T[perm[k]*128 + p, n (global col)]
        AT = np.ascontiguousarray(
            Ab.reshape(K_TILES, 128, NP)[perm].transpose(1, 0, 2))
        b1_host = np.asarray(b1, dtype=f32).reshape(128, 1).copy()
        b2_host = np.asarray(b2, dtype=f32).reshape(128, 1).copy()
        e10 = np.zeros((128, M_TILES, M_TILES), dtype=fp16)
        for j in range(M_TILES):
            e10[:, j, j] = 1.0
    else:
        # [k, p, cm, n] -> [cm, p, k, n], then permute k to chunked layout
        T = Ab.reshape(K_TILES, 128, K_TILES, 128).transpose(2, 1, 0, 3)
        T = T[:, :, _k_perm(), :]
        b1_host = np.ascontiguousarray(
            np.broadcast_to(np.asarray(b1, dtype=f32), (128, H)))
        b2_host = np.ascontiguousarray(
            np.broadcast_to(np.asarray(b2, dtype=f32), (128, H)))

    in_maps = []
    for c in range(N_CORES):
        cols = slice(c * PER_CORE, (c + 1) * PER_CORE)
        xt_c = np.ascontiguousarray(
            XT[:, cols].reshape(2, 128, PER_CORE).transpose(1, 0, 2))
        m = {
            "xt": xt_c,
            "w1": w1_host,
            "w2": w2_host,
            "b1": b1_host,
            "b2": b2_host,
            "ident": ident,
        }
        if dr:
            m["a_t"] = np.ascontiguousarray(AT[:, :, cols])
            m["ones10"] = e10
        else:
            m["a_pre"] = np.ascontiguousarray(
                T[c * M_TILES:(c + 1) * M_TILES])
        in_maps.append(m)
    return in_maps


def _get_nc(agg_mode=None):
    key = f"nc_{agg_mode or AGG_MODE}"
    if key not in _CACHE:
        _CACHE[key] = _build_nc(agg_mode or AGG_MODE)
    return _CACHE[key]


def kernel(X, A, W1, b1, W2, b2, _trace=False, _trace_kwargs=None):
    nc = _get_nc()
    in_maps = _prep_inputs(X, A, W1, b1, W2, b2, AGG_MODE)
    kw = {}
    if _trace:
        kw.update(trace=True, **(_trace_kwargs or {}))
    res = bass_utils.run_bass_kernel_spmd(
        nc, in_maps, core_ids=list(range(N_CORES)), **kw)
    _CACHE["last_result"] = res
    out = np.concatenate([res.results[c]["out"] for c in range(N_CORES)],
                         axis=0)[:N]
    return np.ascontiguousarray(out.astype(np.float32))
